# revision 39
# baseline (speedup 1.0000x reference)
"""Radix-2 DIF ambiguity surface, fp8 DoubleRow DFT + bf16 lag products.

X[k, 2t]   = sum_{m<512} (R[k,m]+R[k,m+512]) * w512^{mt}
X[k, 2t+1] = sum_{m<512} (R[k,m]-R[k,m+512]) * w^m * w512^{mt}

The 512-point DFTs run as fp8e4m3 DoubleRow matmuls (contraction 256 per
instruction: out = W[:,0].T@X[:,0] + W[:,1].T@X[:,1], 0.5 cyc/row), so the
R chunk pairs (q, q+2) live in one [128, 2, 640] tile and the DFT tables in
[128, 2, 6*512] paired tiles. Lag products are built on the DVE from bf16
sliding-window tiles as pure tensor_scalar/tensor_tensor ops (the
scalar_tensor_tensor form with an fp32 scalar ran at <1x). Normalization
uses chi_max = chi(0,0) = (sum |s|^2)^2, computed on-device from the window
tiles (each partition holds a full period, so a free-axis square-accumulate
gives the global sum) and folded into the |X|^2 squares as (alpha*x)^2.
k-mirror via f32r J-matmul on chi; f-mirror rides the PSUM->SBUF copies.
Dummy matmuls during the load/rbuild phase hold the PE HAM at K=8/8.
"""

import numpy as np
import ml_dtypes

import bass_rust
import concourse.bass as bass
import concourse.mybir as mybir
import concourse.tile as tile
import concourse.bass_utils as bass_utils

B, N = 16, 1024
NCORES = 8
BPC = B // NCORES
NKB = 5  # k-blocks: k in [0, 640)
DS_LEN = 2176
W = 1544  # window block width (backward reads start 4B-aligned from base 384)

f32 = mybir.dt.float32
f32r = mybir.dt.float32r
bf16 = mybir.dt.bfloat16
fp8 = mybir.dt.float8e4
ALU = mybir.AluOpType
ACTF = mybir.ActivationFunctionType
PM = mybir.MatmulPerfMode


def _split_excess_waits(nc):
    for f in nc.m.functions:
        for blk in f.blocks:
            insts = list(blk.instructions)
            new_insts = []
            changed = False
            for inst in insts:
                si = inst.sync_info
                waits = list(si.on_wait) if (si is not None and si.on_wait) else []
                keep_n = 0 if isinstance(inst, mybir.InstDrain) else 1
                if len(waits) > keep_n:
                    changed = True
                    extra = waits[: len(waits) - keep_n]
                    keep = waits[len(waits) - keep_n:]
                    for w in extra:
                        nop = mybir.InstNoOp(
                            name=nc.get_next_instruction_name(), ins=[], outs=[]
                        )
                        nop.engine = inst.engine
                        nop.sync_info = bass_rust.SyncInfo(on_wait=[w], on_update=[])
                        new_insts.append(nop)
                    inst.sync_info = bass_rust.SyncInfo(
                        on_wait=keep,
                        on_update=list(si.on_update) if si.on_update else [],
                    )
                new_insts.append(inst)
            if changed:
                blk.instructions = new_insts
    return nc


TABNAMES = ["tec", "tes", "tesn", "toc", "tos", "tosn"]


def build_nc():
    nc = bass.Bass("TRN2", target_bir_lowering=False, debug=False)

    ds2 = nc.dram_tensor("ds2", [BPC, 2, DS_LEN], bf16, kind="ExternalInput")
    scols = nc.dram_tensor("scols", [BPC, 128, 24], f32, kind="ExternalInput")
    tabsp = nc.dram_tensor("tabsp", [2, 128, 2, 6 * 512], fp8, kind="ExternalInput")
    jmat = nc.dram_tensor("jmat", [128, 128], f32r, kind="ExternalInput")
    out = nc.dram_tensor("out", [BPC, N, N], f32, kind="ExternalOutput")

    with tile.TileContext(nc) as tc:
        with (
            tc.tile_pool(name="const", bufs=1) as constp,
            tc.tile_pool(name="tp", bufs=1) as tp,
            tc.tile_pool(name="rp", bufs=1) as rp,
            tc.tile_pool(name="tmp", bufs=2) as tmpp,
            tc.tile_pool(name="u", bufs=1) as up,
            tc.tile_pool(name="chi", bufs=1) as chip,
            tc.tile_pool(name="mj", bufs=2) as mjp,
            tc.tile_pool(name="sm", bufs=1) as smp,
            tc.tile_pool(name="ps", bufs=2, space="PSUM") as psp,
        ):
            tJ = constp.tile([128, 128], f32r, tag="jmat")
            nc.scalar.dma_start(tJ[:], jmat[:])
            # paired DFT tables: TTP[qp][:, sub, 512*i:512*(i+1)] holds table i
            # rows for m-chunk (qp + 2*sub)
            TTP = {}
            for qp in range(2):
                t = constp.tile([128, 2, 6 * 512], fp8, tag=f"ttp{qp}")
                TTP[qp] = t
                nc.gpsimd.dma_start(t[:], tabsp[qp])

            def tab(nm, qp):
                i = TABNAMES.index(nm)
                ap = TTP[qp][:]
                return bass.AP(ap.tensor, ap.offset + 512 * i,
                               [ap.ap[0], [6 * 512, 2], [1, 512]])

            warm = psp.tile([128, 1024], f32, tag="xe")

            def emit_load(b):
                s = {"b": b, "chis": [], "R": {}}
                scol = smp.tile([128, 24], f32, tag=f"scol{b}")
                nc.sync.dma_start(scol[:], scols[b])
                s["scol"] = scol
                wsr = tp.tile([128, W], bf16, name=f"wsr{b}", tag=f"wsr{b}")
                wsi = tp.tile([128, W], bf16, name=f"wsi{b}", tag=f"wsi{b}")
                er, ei = (nc.sync, nc.scalar) if b == 0 else (nc.scalar, nc.sync)
                er.dma_start(wsr[:], bass.AP(ds2, (b * 2) * DS_LEN + 384, [[1, 128], [1, W]]))
                ei.dma_start(wsi[:], bass.AP(ds2, (b * 2 + 1) * DS_LEN + 384, [[1, 128], [1, W]]))
                s["ws"] = (wsr, wsi)
                return s

            def emit_warm(s, n):
                # dummy matmuls to hold the PE HAM unthrottled before the real
                # matmul stream starts; reads the window tile, writes a scratch
                # PSUM bank.
                wsr, _ = s["ws"]
                ap = wsr[:]
                l = bass.AP(ap.tensor, ap.offset, [ap.ap[0], [1, 128]])
                r = bass.AP(ap.tensor, ap.offset, [ap.ap[0], [1, 512]])
                for _ in range(n):
                    nc.tensor.matmul(warm[:, 0:512], l, r, start=True, stop=True)

            def emit_keepalive(dep_tile, mov_tile=None):
                # dummy matmul whose stationary reads a freshly-written tile,
                # so it lands spaced through the producing stream and keeps
                # the PE HAM active
                ap = dep_tile[:]
                l = bass.AP(ap.tensor, ap.offset, [ap.ap[0], [1, 128]])
                mp = (mov_tile if mov_tile is not None else TTP[0])[:]
                r = bass.AP(mp.tensor, mp.offset, [mp.ap[0], [1, 512]])
                nc.tensor.matmul(warm[:, 0:512], l, r, start=True, stop=True)

            def emit_alpha_act(b, s):
                # alpha = 1/sum_m |s[m]|^2: every partition of the sliding
                # window holds a full period, so a free-axis square-accumulate
                # over any 1024 columns yields the global sum per partition.
                wsr, wsi = s["ws"]
                scr = tmpp.tile([128, 1024], f32, tag=f"asc{b % 2}")
                accR = smp.tile([128, 1], f32, tag=f"accR{b}")
                accI = smp.tile([128, 1], f32, tag=f"accI{b}")
                for wt, acc in ((wsr, accR), (wsi, accI)):
                    ap = wt[:]
                    src = bass.AP(ap.tensor, ap.offset + 512, [ap.ap[0], [1, 1024]])
                    nc.scalar.activation(scr[:], src, ACTF.Square, accum_out=acc[:])
                s["accs"] = (accR, accI)

            def emit_alpha_dve(b, s):
                accR, accI = s["accs"]
                ssum = smp.tile([128, 1], f32, tag=f"ssum{b}")
                nc.vector.tensor_add(ssum[:], accR[:], accI[:])
                alpha = smp.tile([128, 1], f32, tag=f"alpha{b}")
                nc.vector.reciprocal(alpha[:], ssum[:])
                s["alpha"] = alpha

            def emit_rbuild(s, lo, hi, keepalive=False):
                # R^T[m, kk] = s[m]*conj(s)[(m-kk)%N]; sum/diff of halves m and
                # m+512, written as fp8 into DoubleRow chunk-pair tiles.
                wsr, wsi = s["ws"]
                wapr, wapi = wsr[:], wsi[:]
                scol = s["scol"]
                b = s["b"]
                n = hi - lo
                for q in range(4):
                    us = []
                    for h in (0, 1):
                        c8 = q + 4 * h
                        sr_c = scol[:, c8:c8 + 1]
                        si_c = scol[:, 8 + c8:9 + c8]
                        nsr_c = scol[:, 16 + c8:17 + c8]
                        j0 = 640 + 128 * q + 512 * h - lo
                        w_sr = bass.AP(wapr.tensor, wapr.offset + j0, [wapr.ap[0], [-1, n]])
                        w_si = bass.AP(wapi.tensor, wapi.offset + j0, [wapi.ap[0], [-1, n]])
                        teng = nc.gpsimd if b == 0 else nc.vector
                        a = tmpp.tile([128, 640], bf16, tag="ta")
                        ur = up.tile([128, 640], bf16, tag=f"ur{h}")
                        # Rr = sr_m*sr_win + si_m*si_win
                        teng.tensor_scalar_mul(a[:, 0:n], w_sr, sr_c)
                        nc.vector.scalar_tensor_tensor(
                            ur[:, 0:n], w_si, si_c, a[:, 0:n], op0=ALU.mult, op1=ALU.add)
                        b2 = tmpp.tile([128, 640], bf16, tag="tb")
                        ui = up.tile([128, 640], bf16, tag=f"ui{h}")
                        # Ri = si_m*sr_win - sr_m*si_win
                        teng.tensor_scalar_mul(b2[:, 0:n], w_si, nsr_c)
                        nc.vector.scalar_tensor_tensor(
                            ui[:, 0:n], w_sr, si_c, b2[:, 0:n], op0=ALU.mult, op1=ALU.add)
                        if keepalive:
                            emit_keepalive(ui, mov_tile=wsr)
                        us.append((ur, ui))
                    (u1r, u1i), (u2r, u2i) = us
                    qp, sub = q % 2, q // 2
                    if lo == 0:
                        for nm in ("rsr", "rsi", "rdr", "rdi"):
                            if (nm, qp) not in s["R"]:
                                s["R"][(nm, qp)] = rp.tile(
                                    [128, 2, 640], fp8,
                                    name=f"{nm}{qp}b{b}", tag=f"{nm}{qp}_{b % 2}",
                                )
                    def rsl(nm):
                        ap = s["R"][(nm, qp)][:]
                        return bass.AP(ap.tensor, ap.offset + sub * 640 + lo, [ap.ap[0], [1, n]])
                    nc.vector.tensor_add(rsl("rsr"), u1r[:, 0:n], u2r[:, 0:n])
                    nc.vector.tensor_sub(rsl("rdr"), u1r[:, 0:n], u2r[:, 0:n])
                    nc.vector.tensor_add(rsl("rsi"), u1i[:, 0:n], u2i[:, 0:n])
                    nc.vector.tensor_sub(rsl("rdi"), u1i[:, 0:n], u2i[:, 0:n])
                    if keepalive:
                        emit_keepalive(s["R"][("rdi", qp)])

            def rweights(s, nm, qp, c):
                ap = s["R"][(nm, qp)][:]
                return bass.AP(ap.tensor, ap.offset + c, [ap.ap[0], [640, 2], [1, 128]])

            def emit_kblock(b, s, kb):
                c = 128 * kb
                xe = psp.tile([128, 1024], f32, tag="xe")
                xo = psp.tile([128, 1024], f32, tag="xo")
                xre, xie = xe[:, 0:512], xe[:, 512:1024]
                xro, xio = xo[:, 0:512], xo[:, 512:1024]
                for qp in range(2):
                    first = qp == 0
                    last = qp == 1
                    psr = rweights(s, "rsr", qp, c)
                    psi = rweights(s, "rsi", qp, c)
                    pdr = rweights(s, "rdr", qp, c)
                    pdi = rweights(s, "rdi", qp, c)
                    mm = lambda o, l, r, st, sp: nc.tensor.matmul(
                        o, l, r, start=st, stop=sp, perf_mode=PM.DoubleRow)
                    # xe's groups close first so its square starts while xo's
                    # last matmuls still stream
                    mm(xre, psr, tab("tec", qp), first, False)
                    mm(xie, psi, tab("tec", qp), first, False)
                    mm(xre, psi, tab("tes", qp), False, last)
                    mm(xie, psr, tab("tesn", qp), False, last)
                    mm(xro, pdr, tab("toc", qp), first, False)
                    mm(xio, pdi, tab("toc", qp), first, False)
                    mm(xro, pdi, tab("tos", qp), False, last)
                    mm(xio, pdr, tab("tosn", qp), False, last)

                chi_t = chip.tile([128, N], f32r, tag=f"chi{(5 * b + kb) % 6}")
                alpha = s["alpha"]
                for parity, x2 in ((0, xe), (1, xo)):
                    sq = tmpp.tile([128, 1024], f32, tag=f"sq{parity}")
                    # chi = (alpha*xr)^2 + (alpha*xi)^2: normalization folded
                    # into the activation scale
                    nc.scalar.activation(sq[:], x2[:], ACTF.Square, scale=alpha[:])
                    cap = chi_t[:]
                    strided = bass.AP(cap.tensor, cap.offset + parity, [cap.ap[0], [2, 512]])
                    aeng = nc.gpsimd if b == 0 else nc.vector
                    aeng.tensor_add(strided, sq[:, 0:512], sq[:, 512:1024])
                s["chis"].append(chi_t)

            def emit_direct(b, s, kbs):
                for kb in kbs:
                    r0 = (128 * kb + 512) % N
                    ch = s["chis"][kb][:].bitcast(f32)
                    if b == 1 and kb == 4:
                        nc.sync.dma_start(out[b, r0:r0 + 64, :], s["chis"][kb][0:64, :].bitcast(f32))
                        nc.gpsimd.dma_start(out[b, r0 + 64:r0 + 128, :], s["chis"][kb][64:128, :].bitcast(f32))
                    else:
                        nc.sync.dma_start(out[b, r0:r0 + 128, :], ch)

            def emit_mirror_jcopy(b, s, kbs):
                # k-flip: J matmul on chi reverses partitions; the f-reversal
                # rides the PSUM->SBUF copies. ACT handles batch 0 (DVE busy
                # with rbuild(s1)); DVE handles batch 1 (idle by then).
                s.setdefault("mj", {})
                cp = nc.scalar.copy
                for kb in kbs:
                    chi_t = s["chis"][kb]
                    jy = psp.tile([128, 1024], f32, tag="xe")
                    nc.tensor.matmul(jy[:, 0:512], tJ[:],
                                     chi_t[:, 0:512], start=True, stop=True)
                    nc.tensor.matmul(jy[:, 512:1024], tJ[:],
                                     chi_t[:, 512:1024], start=True, stop=True)
                    mj = mjp.tile([128, N], f32, tag=f"mj{kb % 2}")
                    jap = jy[:]
                    rev_hi = bass.AP(jap.tensor, jap.offset + 1023, [jap.ap[0], [-1, 511]])
                    rev_lo = bass.AP(jap.tensor, jap.offset + 511, [jap.ap[0], [-1, 511]])
                    cp(mj[:, 0:1], jy[:, 0:1])
                    cp(mj[:, 1:512], rev_hi)
                    cp(mj[:, 512:513], jy[:, 512:513])
                    cp(mj[:, 513:1024], rev_lo)
                    s["mj"][kb] = mj

            def emit_mirror_store(b, s, kbs):
                # mj partition r holds k = c+127-r -> dest row 385-c+r
                for kb in kbs:
                    c = 128 * kb
                    mj = s["mj"][kb]
                    eng = nc.scalar
                    if kb == 0:
                        eng.dma_start(out[b, 385:512, :], mj[0:127, :])
                    elif kb == 3:
                        eng.dma_start(out[b, 128:129, :], mj[127:128, :])
                    else:
                        r0 = 385 - c
                        if b == 1 and kb == 2:
                            # final mirror: split across rings so the tail
                            # drains in parallel
                            nc.scalar.dma_start(out[b, r0:r0 + 64, :], mj[0:64, :])
                            nc.gpsimd.dma_start(out[b, r0 + 64:r0 + 128, :], mj[64:128, :])
                        else:
                            eng.dma_start(out[b, r0:r0 + 128, :], mj[:])

            # --- pipelined schedule: DVE runs the two rbuilds back-to-back;
            # the PE streams batch-0 kblocks against rbuild(s1); warm/keepalive
            # matmuls bridge the PE-idle stretches so the HAM stays at 8/8.
            s0 = emit_load(0)
            s1 = emit_load(1)
            emit_warm(s0, 14)
            emit_alpha_act(0, s0)
            emit_alpha_act(1, s1)
            emit_rbuild(s0, 0, 640, keepalive=True)
            emit_alpha_dve(0, s0)
            emit_kblock(0, s0, 0)
            emit_direct(0, s0, [0])
            emit_alpha_dve(1, s1)
            emit_rbuild(s1, 0, 640)
            emit_kblock(0, s0, 1)
            emit_direct(0, s0, [1])
            emit_mirror_jcopy(0, s0, [0])
            emit_mirror_store(0, s0, [0])
            emit_kblock(0, s0, 2)
            emit_direct(0, s0, [2])
            emit_mirror_jcopy(0, s0, [1])
            emit_mirror_store(0, s0, [1])
            emit_kblock(0, s0, 3)
            emit_direct(0, s0, [3])
            emit_mirror_jcopy(0, s0, [2])
            emit_mirror_store(0, s0, [2])
            emit_kblock(0, s0, 4)
            emit_direct(0, s0, [4])
            emit_mirror_jcopy(0, s0, [3])
            emit_mirror_store(0, s0, [3])
            # bridge the gap until rbuild(s1) completes, one keepalive per
            # R array so the deps land spread across its combine stream
            for nm in ("rsr", "rsi", "rdr", "rdi"):
                ap = s1["R"][(nm, 1)][:]
                l = bass.AP(ap.tensor, ap.offset, [ap.ap[0], [1, 128]])
                tp_ap = TTP[0][:]
                r = bass.AP(tp_ap.tensor, tp_ap.offset, [tp_ap.ap[0], [1, 512]])
                nc.tensor.matmul(warm[:, 0:512], l, r, start=True, stop=True)
            emit_kblock(1, s1, 0)
            emit_direct(1, s1, [0])
            for kb in range(1, 4):
                emit_kblock(1, s1, kb)
                emit_direct(1, s1, [kb])
                emit_mirror_jcopy(1, s1, [kb - 1])
                emit_mirror_store(1, s1, [kb - 1])
            emit_kblock(1, s1, 4)
            emit_direct(1, s1, [4])
            emit_mirror_jcopy(1, s1, [3])
            emit_mirror_store(1, s1, [3])

    _split_excess_waits(nc)
    return nc


_NC_CACHE = {}


def _get_nc():
    if "nc" not in _NC_CACHE:
        _NC_CACHE["nc"] = build_nc()
    return _NC_CACHE["nc"]


def _get_tables():
    if "tabs" not in _NC_CACHE:
        m = np.arange(512, dtype=np.float64)[:, None]
        tp_ = np.arange(512, dtype=np.float64)[None, :]
        t_of = (tp_ + 256) % 512
        ang_e = 2.0 * np.pi * ((m * t_of) % 512) / 512
        ang_o = ang_e + 2.0 * np.pi * m / 1024
        tabs = {
            "tec": np.cos(ang_e),
            "tes": np.sin(ang_e),
            "toc": np.cos(ang_o),
            "tos": np.sin(ang_o),
        }
        tabs["tesn"] = -tabs["tes"]
        tabs["tosn"] = -tabs["tos"]
        # paired fp8 layout: tabsp[qp, p, sub, 512*i+t] = tab_i[128*(qp+2*sub)+p, t]
        tabsp = np.zeros((2, 128, 2, 6 * 512), dtype=np.float64)
        for i, nm in enumerate(TABNAMES):
            tq = tabs[nm].reshape(4, 128, 512)  # [chunk, p, t]
            for qp in range(2):
                for sub in range(2):
                    tabsp[qp, :, sub, 512 * i:512 * (i + 1)] = tq[qp + 2 * sub]
        _NC_CACHE["tabs"] = (
            tabsp.astype(ml_dtypes.float8_e4m3),
            np.eye(128, dtype=np.float32)[::-1].copy(),
        )
    return _NC_CACHE["tabs"]


def make_in_maps(s_real: np.ndarray, s_imag: np.ndarray):
    s_real = np.asarray(s_real, dtype=np.float32)
    s_imag = np.asarray(s_imag, dtype=np.float32)
    tabsp, jnp_ = _get_tables()
    in_maps = []
    for core in range(NCORES):
        sl = slice(core * BPC, (core + 1) * BPC)
        sr = s_real[sl].astype(ml_dtypes.bfloat16)
        si = s_imag[sl].astype(ml_dtypes.bfloat16)
        ds2 = np.stack(
            [np.tile(sr, (1, 3))[:, :DS_LEN], np.tile(si, (1, 3))[:, :DS_LEN]],
            axis=1,
        ).copy()
        scols = np.concatenate(
            [
                sr.reshape(BPC, 8, 128).transpose(0, 2, 1),
                si.reshape(BPC, 8, 128).transpose(0, 2, 1),
                (-sr).reshape(BPC, 8, 128).transpose(0, 2, 1),
            ],
            axis=2,
        ).astype(np.float32).copy()
        im = {"ds2": ds2, "scols": scols, "tabsp": tabsp, "jmat": jnp_}
        in_maps.append(im)
    return in_maps


def kernel(s_real: np.ndarray, s_imag: np.ndarray) -> np.ndarray:
    nc = _get_nc()
    in_maps = make_in_maps(s_real, s_imag)
    res = bass_utils.run_bass_kernel_spmd(nc, in_maps, core_ids=list(range(NCORES)))
    return np.concatenate([r["out"] for r in res.results], axis=0)


# revision 40
# speedup vs baseline: 1.9674x; 1.9674x over previous
"""Radix-2 DIF ambiguity surface, fp8 DoubleRow DFT + bf16 lag products.

X[k, 2t]   = sum_{m<512} (R[k,m]+R[k,m+512]) * w512^{mt}
X[k, 2t+1] = sum_{m<512} (R[k,m]-R[k,m+512]) * w^m * w512^{mt}

The 512-point DFTs run as fp8e4m3 DoubleRow matmuls (contraction 256 per
instruction: out = W[:,0].T@X[:,0] + W[:,1].T@X[:,1], 0.5 cyc/row), so the
R chunk pairs (q, q+2) live in one [128, 2, 640] tile and the DFT tables in
[128, 2, 6*512] paired tiles. Lag products are built on the DVE from bf16
sliding-window tiles as pure tensor_scalar/tensor_tensor ops (the
scalar_tensor_tensor form with an fp32 scalar ran at <1x). Normalization
uses chi_max = chi(0,0) = (sum |s|^2)^2, computed on-device from the window
tiles (each partition holds a full period, so a free-axis square-accumulate
gives the global sum) and folded into the |X|^2 squares as (alpha*x)^2.
k-mirror via f32r J-matmul on chi; f-mirror rides the PSUM->SBUF copies.
Dummy matmuls during the load/rbuild phase hold the PE HAM at K=8/8.
"""

import numpy as np
import ml_dtypes

import bass_rust
import concourse.bass as bass
import concourse.mybir as mybir
import concourse.tile as tile
import concourse.bass_utils as bass_utils

B, N = 16, 1024
NCORES = 8
BPC = B // NCORES
NKB = 5  # k-blocks: k in [0, 640)
DS_LEN = 2176
W = 1544  # window block width (backward reads start 4B-aligned from base 384)

f32 = mybir.dt.float32
f32r = mybir.dt.float32r
bf16 = mybir.dt.bfloat16
fp8 = mybir.dt.float8e4
ALU = mybir.AluOpType
ACTF = mybir.ActivationFunctionType
PM = mybir.MatmulPerfMode


def _split_excess_waits(nc):
    for f in nc.m.functions:
        for blk in f.blocks:
            insts = list(blk.instructions)
            new_insts = []
            changed = False
            for inst in insts:
                si = inst.sync_info
                waits = list(si.on_wait) if (si is not None and si.on_wait) else []
                keep_n = 0 if isinstance(inst, mybir.InstDrain) else 1
                if len(waits) > keep_n:
                    changed = True
                    extra = waits[: len(waits) - keep_n]
                    keep = waits[len(waits) - keep_n:]
                    for w in extra:
                        nop = mybir.InstNoOp(
                            name=nc.get_next_instruction_name(), ins=[], outs=[]
                        )
                        nop.engine = inst.engine
                        nop.sync_info = bass_rust.SyncInfo(on_wait=[w], on_update=[])
                        new_insts.append(nop)
                    inst.sync_info = bass_rust.SyncInfo(
                        on_wait=keep,
                        on_update=list(si.on_update) if si.on_update else [],
                    )
                new_insts.append(inst)
            if changed:
                blk.instructions = new_insts
    return nc


TABNAMES = ["tec", "tes", "tesn", "toc", "tos", "tosn"]


def build_nc():
    nc = bass.Bass("TRN2", target_bir_lowering=False, debug=False)

    ds2 = nc.dram_tensor("ds2", [BPC, 2, DS_LEN], bf16, kind="ExternalInput")
    scols = nc.dram_tensor("scols", [BPC, 128, 24], f32, kind="ExternalInput")
    tabsp = nc.dram_tensor("tabsp", [2, 128, 2, 6 * 512], fp8, kind="ExternalInput")
    jmat = nc.dram_tensor("jmat", [128, 128], f32r, kind="ExternalInput")
    out = nc.dram_tensor("out", [BPC, N, N], f32, kind="ExternalOutput")

    with tile.TileContext(nc) as tc:
        with (
            tc.tile_pool(name="const", bufs=1) as constp,
            tc.tile_pool(name="tp", bufs=1) as tp,
            tc.tile_pool(name="rp", bufs=1) as rp,
            tc.tile_pool(name="tmp", bufs=2) as tmpp,
            tc.tile_pool(name="u", bufs=1) as up,
            tc.tile_pool(name="chi", bufs=1) as chip,
            tc.tile_pool(name="mj", bufs=2) as mjp,
            tc.tile_pool(name="sm", bufs=1) as smp,
            tc.tile_pool(name="ps", bufs=2, space="PSUM") as psp,
        ):
            tJ = constp.tile([128, 128], f32r, tag="jmat")
            nc.scalar.dma_start(tJ[:], jmat[:])
            # paired DFT tables: TTP[qp][:, sub, 512*i:512*(i+1)] holds table i
            # rows for m-chunk (qp + 2*sub)
            TTP = {}
            for qp in range(2):
                t = constp.tile([128, 2, 6 * 512], fp8, tag=f"ttp{qp}")
                TTP[qp] = t
                nc.gpsimd.dma_start(t[:], tabsp[qp])

            def tab(nm, qp):
                i = TABNAMES.index(nm)
                ap = TTP[qp][:]
                return bass.AP(ap.tensor, ap.offset + 512 * i,
                               [ap.ap[0], [6 * 512, 2], [1, 512]])

            warm = psp.tile([128, 1024], f32, tag="xe")

            def emit_load(b):
                s = {"b": b, "chis": [], "R": {}}
                scol = smp.tile([128, 24], f32, tag=f"scol{b}")
                nc.sync.dma_start(scol[:], scols[b])
                s["scol"] = scol
                wsr = tp.tile([128, W], bf16, name=f"wsr{b}", tag=f"wsr{b}")
                wsi = tp.tile([128, W], bf16, name=f"wsi{b}", tag=f"wsi{b}")
                er, ei = (nc.sync, nc.scalar) if b == 0 else (nc.scalar, nc.sync)
                er.dma_start(wsr[:], bass.AP(ds2, (b * 2) * DS_LEN + 384, [[1, 128], [1, W]]))
                ei.dma_start(wsi[:], bass.AP(ds2, (b * 2 + 1) * DS_LEN + 384, [[1, 128], [1, W]]))
                s["ws"] = (wsr, wsi)
                return s

            def emit_warm(s, n):
                # dummy matmuls to hold the PE HAM unthrottled before the real
                # matmul stream starts; reads the window tile, writes a scratch
                # PSUM bank.
                wsr, _ = s["ws"]
                ap = wsr[:]
                l = bass.AP(ap.tensor, ap.offset, [ap.ap[0], [1, 128]])
                r = bass.AP(ap.tensor, ap.offset, [ap.ap[0], [1, 512]])
                for _ in range(n):
                    nc.tensor.matmul(warm[:, 0:512], l, r, start=True, stop=True)

            def emit_keepalive(dep_tile, mov_tile=None):
                # dummy matmul whose stationary reads a freshly-written tile,
                # so it lands spaced through the producing stream and keeps
                # the PE HAM active
                ap = dep_tile[:]
                l = bass.AP(ap.tensor, ap.offset, [ap.ap[0], [1, 128]])
                mp = (mov_tile if mov_tile is not None else TTP[0])[:]
                r = bass.AP(mp.tensor, mp.offset, [mp.ap[0], [1, 512]])
                nc.tensor.matmul(warm[:, 0:512], l, r, start=True, stop=True)

            def emit_alpha_act(b, s):
                # alpha = 1/sum_m |s[m]|^2: every partition of the sliding
                # window holds a full period, so a free-axis square-accumulate
                # over any 1024 columns yields the global sum per partition.
                wsr, wsi = s["ws"]
                scr = tmpp.tile([128, 1024], f32, tag=f"asc{b % 2}")
                accR = smp.tile([128, 1], f32, tag=f"accR{b}")
                accI = smp.tile([128, 1], f32, tag=f"accI{b}")
                for wt, acc in ((wsr, accR), (wsi, accI)):
                    ap = wt[:]
                    src = bass.AP(ap.tensor, ap.offset + 512, [ap.ap[0], [1, 1024]])
                    nc.scalar.activation(scr[:], src, ACTF.Square, accum_out=acc[:])
                s["accs"] = (accR, accI)

            def emit_alpha_dve(b, s):
                accR, accI = s["accs"]
                ssum = smp.tile([128, 1], f32, tag=f"ssum{b}")
                nc.vector.tensor_add(ssum[:], accR[:], accI[:])
                alpha = smp.tile([128, 1], f32, tag=f"alpha{b}")
                nc.vector.reciprocal(alpha[:], ssum[:])
                s["alpha"] = alpha

            def emit_rbuild(s, lo, hi, keepalive=False):
                # R^T[m, kk] = s[m]*conj(s)[(m-kk)%N]; sum/diff of halves m and
                # m+512, written as fp8 into DoubleRow chunk-pair tiles.
                wsr, wsi = s["ws"]
                wapr, wapi = wsr[:], wsi[:]
                scol = s["scol"]
                b = s["b"]
                n = hi - lo
                for q in range(4):
                    us = []
                    for h in (0, 1):
                        c8 = q + 4 * h
                        sr_c = scol[:, c8:c8 + 1]
                        si_c = scol[:, 8 + c8:9 + c8]
                        nsr_c = scol[:, 16 + c8:17 + c8]
                        j0 = 640 + 128 * q + 512 * h - lo
                        w_sr = bass.AP(wapr.tensor, wapr.offset + j0, [wapr.ap[0], [-1, n]])
                        w_si = bass.AP(wapi.tensor, wapi.offset + j0, [wapi.ap[0], [-1, n]])
                        teng = nc.vector
                        a = tmpp.tile([128, 640], bf16, tag="ta")
                        ur = up.tile([128, 640], bf16, tag=f"ur{h}")
                        # Rr = sr_m*sr_win + si_m*si_win
                        teng.tensor_scalar_mul(a[:, 0:n], w_sr, sr_c)
                        nc.vector.scalar_tensor_tensor(
                            ur[:, 0:n], w_si, si_c, a[:, 0:n], op0=ALU.mult, op1=ALU.add)
                        b2 = tmpp.tile([128, 640], bf16, tag="tb")
                        ui = up.tile([128, 640], bf16, tag=f"ui{h}")
                        # Ri = si_m*sr_win - sr_m*si_win
                        teng.tensor_scalar_mul(b2[:, 0:n], w_si, nsr_c)
                        nc.vector.scalar_tensor_tensor(
                            ui[:, 0:n], w_sr, si_c, b2[:, 0:n], op0=ALU.mult, op1=ALU.add)
                        if keepalive:
                            emit_keepalive(ui, mov_tile=wsr)
                        us.append((ur, ui))
                    (u1r, u1i), (u2r, u2i) = us
                    qp, sub = q % 2, q // 2
                    if lo == 0:
                        for nm in ("rsr", "rsi", "rdr", "rdi"):
                            if (nm, qp) not in s["R"]:
                                s["R"][(nm, qp)] = rp.tile(
                                    [128, 2, 640], fp8,
                                    name=f"{nm}{qp}b{b}", tag=f"{nm}{qp}_{b % 2}",
                                )
                    def rsl(nm):
                        ap = s["R"][(nm, qp)][:]
                        return bass.AP(ap.tensor, ap.offset + sub * 640 + lo, [ap.ap[0], [1, n]])
                    nc.vector.tensor_add(rsl("rsr"), u1r[:, 0:n], u2r[:, 0:n])
                    nc.vector.tensor_sub(rsl("rdr"), u1r[:, 0:n], u2r[:, 0:n])
                    nc.vector.tensor_add(rsl("rsi"), u1i[:, 0:n], u2i[:, 0:n])
                    nc.vector.tensor_sub(rsl("rdi"), u1i[:, 0:n], u2i[:, 0:n])
                    if keepalive:
                        emit_keepalive(s["R"][("rdi", qp)])

            def rweights(s, nm, qp, c):
                ap = s["R"][(nm, qp)][:]
                return bass.AP(ap.tensor, ap.offset + c, [ap.ap[0], [640, 2], [1, 128]])

            def emit_kblock(b, s, kb):
                c = 128 * kb
                xe = psp.tile([128, 1024], f32, tag="xe")
                xo = psp.tile([128, 1024], f32, tag="xo")
                xre, xie = xe[:, 0:512], xe[:, 512:1024]
                xro, xio = xo[:, 0:512], xo[:, 512:1024]
                for qp in range(2):
                    first = qp == 0
                    last = qp == 1
                    psr = rweights(s, "rsr", qp, c)
                    psi = rweights(s, "rsi", qp, c)
                    pdr = rweights(s, "rdr", qp, c)
                    pdi = rweights(s, "rdi", qp, c)
                    mm = lambda o, l, r, st, sp: nc.tensor.matmul(
                        o, l, r, start=st, stop=sp, perf_mode=PM.DoubleRow)
                    # xe's groups close first so its square starts while xo's
                    # last matmuls still stream
                    mm(xre, psr, tab("tec", qp), first, False)
                    mm(xie, psi, tab("tec", qp), first, False)
                    mm(xre, psi, tab("tes", qp), False, last)
                    mm(xie, psr, tab("tesn", qp), False, last)
                    mm(xro, pdr, tab("toc", qp), first, False)
                    mm(xio, pdi, tab("toc", qp), first, False)
                    mm(xro, pdi, tab("tos", qp), False, last)
                    mm(xio, pdr, tab("tosn", qp), False, last)

                chi_t = chip.tile([128, N], f32r, tag=f"chi{(5 * b + kb) % 6}")
                alpha = s["alpha"]
                for parity, x2 in ((0, xe), (1, xo)):
                    sq = tmpp.tile([128, 1024], f32, tag=f"sq{parity}")
                    # chi = (alpha*xr)^2 + (alpha*xi)^2: normalization folded
                    # into the activation scale
                    nc.scalar.activation(sq[:], x2[:], ACTF.Square, scale=alpha[:])
                    cap = chi_t[:]
                    strided = bass.AP(cap.tensor, cap.offset + parity, [cap.ap[0], [2, 512]])
                    aeng = nc.gpsimd if b == 0 else nc.vector
                    aeng.tensor_add(strided, sq[:, 0:512], sq[:, 512:1024])
                s["chis"].append(chi_t)

            def emit_direct(b, s, kbs):
                for kb in kbs:
                    r0 = (128 * kb + 512) % N
                    ch = s["chis"][kb][:].bitcast(f32)
                    if b == 1 and kb == 4:
                        nc.sync.dma_start(out[b, r0:r0 + 64, :], s["chis"][kb][0:64, :].bitcast(f32))
                        nc.gpsimd.dma_start(out[b, r0 + 64:r0 + 128, :], s["chis"][kb][64:128, :].bitcast(f32))
                    else:
                        nc.sync.dma_start(out[b, r0:r0 + 128, :], ch)

            def emit_mirror_jcopy(b, s, kbs):
                # k-flip: J matmul on chi reverses partitions; the f-reversal
                # rides the PSUM->SBUF copies. ACT handles batch 0 (DVE busy
                # with rbuild(s1)); DVE handles batch 1 (idle by then).
                s.setdefault("mj", {})
                cp = nc.scalar.copy
                for kb in kbs:
                    chi_t = s["chis"][kb]
                    jy = psp.tile([128, 1024], f32, tag="xe")
                    nc.tensor.matmul(jy[:, 0:512], tJ[:],
                                     chi_t[:, 0:512], start=True, stop=True)
                    nc.tensor.matmul(jy[:, 512:1024], tJ[:],
                                     chi_t[:, 512:1024], start=True, stop=True)
                    mj = mjp.tile([128, N], f32, tag=f"mj{kb % 2}")
                    jap = jy[:]
                    rev_hi = bass.AP(jap.tensor, jap.offset + 1023, [jap.ap[0], [-1, 511]])
                    rev_lo = bass.AP(jap.tensor, jap.offset + 511, [jap.ap[0], [-1, 511]])
                    cp(mj[:, 0:1], jy[:, 0:1])
                    cp(mj[:, 1:512], rev_hi)
                    cp(mj[:, 512:513], jy[:, 512:513])
                    cp(mj[:, 513:1024], rev_lo)
                    s["mj"][kb] = mj

            def emit_mirror_store(b, s, kbs):
                # mj partition r holds k = c+127-r -> dest row 385-c+r
                for kb in kbs:
                    c = 128 * kb
                    mj = s["mj"][kb]
                    eng = nc.scalar
                    if kb == 0:
                        eng.dma_start(out[b, 385:512, :], mj[0:127, :])
                    elif kb == 3:
                        eng.dma_start(out[b, 128:129, :], mj[127:128, :])
                    else:
                        r0 = 385 - c
                        if b == 1 and kb == 2:
                            # final mirror: split across rings so the tail
                            # drains in parallel
                            nc.scalar.dma_start(out[b, r0:r0 + 64, :], mj[0:64, :])
                            nc.gpsimd.dma_start(out[b, r0 + 64:r0 + 128, :], mj[64:128, :])
                        else:
                            eng.dma_start(out[b, r0:r0 + 128, :], mj[:])

            # --- pipelined schedule: DVE runs the two rbuilds back-to-back;
            # the PE streams batch-0 kblocks against rbuild(s1); warm/keepalive
            # matmuls bridge the PE-idle stretches so the HAM stays at 8/8.
            s0 = emit_load(0)
            s1 = emit_load(1)
            emit_warm(s0, 14)
            emit_alpha_act(0, s0)
            emit_alpha_act(1, s1)
            emit_rbuild(s0, 0, 640, keepalive=True)
            emit_alpha_dve(0, s0)
            emit_kblock(0, s0, 0)
            emit_direct(0, s0, [0])
            emit_alpha_dve(1, s1)
            emit_rbuild(s1, 0, 640)
            emit_kblock(0, s0, 1)
            emit_direct(0, s0, [1])
            emit_mirror_jcopy(0, s0, [0])
            emit_mirror_store(0, s0, [0])
            emit_kblock(0, s0, 2)
            emit_direct(0, s0, [2])
            emit_mirror_jcopy(0, s0, [1])
            emit_mirror_store(0, s0, [1])
            emit_kblock(0, s0, 3)
            emit_direct(0, s0, [3])
            emit_mirror_jcopy(0, s0, [2])
            emit_mirror_store(0, s0, [2])
            emit_kblock(0, s0, 4)
            emit_direct(0, s0, [4])
            emit_mirror_jcopy(0, s0, [3])
            emit_mirror_store(0, s0, [3])
            # bridge the gap until rbuild(s1) completes, one keepalive per
            # R array so the deps land spread across its combine stream
            for nm in ("rsr", "rsi", "rdr", "rdi"):
                ap = s1["R"][(nm, 1)][:]
                l = bass.AP(ap.tensor, ap.offset, [ap.ap[0], [1, 128]])
                tp_ap = TTP[0][:]
                r = bass.AP(tp_ap.tensor, tp_ap.offset, [tp_ap.ap[0], [1, 512]])
                nc.tensor.matmul(warm[:, 0:512], l, r, start=True, stop=True)
            emit_kblock(1, s1, 0)
            emit_direct(1, s1, [0])
            for kb in range(1, 4):
                emit_kblock(1, s1, kb)
                emit_direct(1, s1, [kb])
                emit_mirror_jcopy(1, s1, [kb - 1])
                emit_mirror_store(1, s1, [kb - 1])
            emit_kblock(1, s1, 4)
            emit_direct(1, s1, [4])
            emit_mirror_jcopy(1, s1, [3])
            emit_mirror_store(1, s1, [3])

    _split_excess_waits(nc)
    return nc


_NC_CACHE = {}


def _get_nc():
    if "nc" not in _NC_CACHE:
        _NC_CACHE["nc"] = build_nc()
    return _NC_CACHE["nc"]


def _get_tables():
    if "tabs" not in _NC_CACHE:
        m = np.arange(512, dtype=np.float64)[:, None]
        tp_ = np.arange(512, dtype=np.float64)[None, :]
        t_of = (tp_ + 256) % 512
        ang_e = 2.0 * np.pi * ((m * t_of) % 512) / 512
        ang_o = ang_e + 2.0 * np.pi * m / 1024
        tabs = {
            "tec": np.cos(ang_e),
            "tes": np.sin(ang_e),
            "toc": np.cos(ang_o),
            "tos": np.sin(ang_o),
        }
        tabs["tesn"] = -tabs["tes"]
        tabs["tosn"] = -tabs["tos"]
        # paired fp8 layout: tabsp[qp, p, sub, 512*i+t] = tab_i[128*(qp+2*sub)+p, t]
        tabsp = np.zeros((2, 128, 2, 6 * 512), dtype=np.float64)
        for i, nm in enumerate(TABNAMES):
            tq = tabs[nm].reshape(4, 128, 512)  # [chunk, p, t]
            for qp in range(2):
                for sub in range(2):
                    tabsp[qp, :, sub, 512 * i:512 * (i + 1)] = tq[qp + 2 * sub]
        _NC_CACHE["tabs"] = (
            tabsp.astype(ml_dtypes.float8_e4m3),
            np.eye(128, dtype=np.float32)[::-1].copy(),
        )
    return _NC_CACHE["tabs"]


def make_in_maps(s_real: np.ndarray, s_imag: np.ndarray):
    s_real = np.asarray(s_real, dtype=np.float32)
    s_imag = np.asarray(s_imag, dtype=np.float32)
    tabsp, jnp_ = _get_tables()
    in_maps = []
    for core in range(NCORES):
        sl = slice(core * BPC, (core + 1) * BPC)
        sr = s_real[sl].astype(ml_dtypes.bfloat16)
        si = s_imag[sl].astype(ml_dtypes.bfloat16)
        ds2 = np.stack(
            [np.tile(sr, (1, 3))[:, :DS_LEN], np.tile(si, (1, 3))[:, :DS_LEN]],
            axis=1,
        ).copy()
        scols = np.concatenate(
            [
                sr.reshape(BPC, 8, 128).transpose(0, 2, 1),
                si.reshape(BPC, 8, 128).transpose(0, 2, 1),
                (-sr).reshape(BPC, 8, 128).transpose(0, 2, 1),
            ],
            axis=2,
        ).astype(np.float32).copy()
        im = {"ds2": ds2, "scols": scols, "tabsp": tabsp, "jmat": jnp_}
        in_maps.append(im)
    return in_maps


def kernel(s_real: np.ndarray, s_imag: np.ndarray) -> np.ndarray:
    nc = _get_nc()
    in_maps = make_in_maps(s_real, s_imag)
    res = bass_utils.run_bass_kernel_spmd(nc, in_maps, core_ids=list(range(NCORES)))
    return np.concatenate([r["out"] for r in res.results], axis=0)


# revision 41
# speedup vs baseline: 2.0049x; 1.0191x over previous
"""Radix-2 DIF ambiguity surface, fp8 DoubleRow DFT + bf16 lag products.

X[k, 2t]   = sum_{m<512} (R[k,m]+R[k,m+512]) * w512^{mt}
X[k, 2t+1] = sum_{m<512} (R[k,m]-R[k,m+512]) * w^m * w512^{mt}

The 512-point DFTs run as fp8e4m3 DoubleRow matmuls (contraction 256 per
instruction: out = W[:,0].T@X[:,0] + W[:,1].T@X[:,1], 0.5 cyc/row), so the
R chunk pairs (q, q+2) live in one [128, 2, 640] tile and the DFT tables in
[128, 2, 6*512] paired tiles. Lag products are built on the DVE from bf16
sliding-window tiles as pure tensor_scalar/tensor_tensor ops (the
scalar_tensor_tensor form with an fp32 scalar ran at <1x). Normalization
uses chi_max = chi(0,0) = (sum |s|^2)^2, computed on-device from the window
tiles (each partition holds a full period, so a free-axis square-accumulate
gives the global sum) and folded into the |X|^2 squares as (alpha*x)^2.
k-mirror via f32r J-matmul on chi; f-mirror rides the PSUM->SBUF copies.
Dummy matmuls during the load/rbuild phase hold the PE HAM at K=8/8.
"""

import numpy as np
import ml_dtypes

import bass_rust
import concourse.bass as bass
import concourse.mybir as mybir
import concourse.tile as tile
import concourse.bass_utils as bass_utils

B, N = 16, 1024
NCORES = 8
BPC = B // NCORES
NKB = 5  # k-blocks: k in [0, 640)
DS_LEN = 2176
W = 1544  # window block width (backward reads start 4B-aligned from base 384)

f32 = mybir.dt.float32
f32r = mybir.dt.float32r
bf16 = mybir.dt.bfloat16
fp8 = mybir.dt.float8e4
ALU = mybir.AluOpType
ACTF = mybir.ActivationFunctionType
PM = mybir.MatmulPerfMode


def _split_excess_waits(nc):
    for f in nc.m.functions:
        for blk in f.blocks:
            insts = list(blk.instructions)
            new_insts = []
            changed = False
            for inst in insts:
                si = inst.sync_info
                waits = list(si.on_wait) if (si is not None and si.on_wait) else []
                keep_n = 0 if isinstance(inst, mybir.InstDrain) else 1
                if len(waits) > keep_n:
                    changed = True
                    extra = waits[: len(waits) - keep_n]
                    keep = waits[len(waits) - keep_n:]
                    for w in extra:
                        nop = mybir.InstNoOp(
                            name=nc.get_next_instruction_name(), ins=[], outs=[]
                        )
                        nop.engine = inst.engine
                        nop.sync_info = bass_rust.SyncInfo(on_wait=[w], on_update=[])
                        new_insts.append(nop)
                    inst.sync_info = bass_rust.SyncInfo(
                        on_wait=keep,
                        on_update=list(si.on_update) if si.on_update else [],
                    )
                new_insts.append(inst)
            if changed:
                blk.instructions = new_insts
    return nc


TABNAMES = ["tec", "tes", "tesn", "toc", "tos", "tosn"]


def build_nc():
    nc = bass.Bass("TRN2", target_bir_lowering=False, debug=False)

    ds2 = nc.dram_tensor("ds2", [BPC, 2, DS_LEN], bf16, kind="ExternalInput")
    scols = nc.dram_tensor("scols", [BPC, 128, 24], f32, kind="ExternalInput")
    tabsp = nc.dram_tensor("tabsp", [2, 128, 2, 6 * 512], fp8, kind="ExternalInput")
    jmat = nc.dram_tensor("jmat", [128, 128], f32r, kind="ExternalInput")
    out = nc.dram_tensor("out", [BPC, N, N], f32, kind="ExternalOutput")

    with tile.TileContext(nc) as tc:
        with (
            tc.tile_pool(name="const", bufs=1) as constp,
            tc.tile_pool(name="tp", bufs=1) as tp,
            tc.tile_pool(name="rp", bufs=1) as rp,
            tc.tile_pool(name="tmp", bufs=2) as tmpp,
            tc.tile_pool(name="u", bufs=1) as up,
            tc.tile_pool(name="chi", bufs=1) as chip,
            tc.tile_pool(name="mj", bufs=2) as mjp,
            tc.tile_pool(name="sm", bufs=1) as smp,
            tc.tile_pool(name="ps", bufs=2, space="PSUM") as psp,
        ):
            tJ = constp.tile([128, 128], f32r, tag="jmat")
            # paired DFT tables: TTP[qp][:, sub, 512*i:512*(i+1)] holds table i
            # rows for m-chunk (qp + 2*sub)
            TTP = {}
            for qp in range(2):
                t = constp.tile([128, 2, 6 * 512], fp8, tag=f"ttp{qp}")
                TTP[qp] = t
                nc.gpsimd.dma_start(t[:], tabsp[qp])

            def tab(nm, qp):
                i = TABNAMES.index(nm)
                ap = TTP[qp][:]
                return bass.AP(ap.tensor, ap.offset + 512 * i,
                               [ap.ap[0], [6 * 512, 2], [1, 512]])

            warm = psp.tile([128, 1024], f32, tag="xe")

            def emit_load(b):
                s = {"b": b, "chis": [], "R": {}}
                scol = smp.tile([128, 24], f32, tag=f"scol{b}")
                nc.sync.dma_start(scol[:], scols[b])
                s["scol"] = scol
                wsr = tp.tile([128, W], bf16, name=f"wsr{b}", tag=f"wsr{b}")
                wsi = tp.tile([128, W], bf16, name=f"wsi{b}", tag=f"wsi{b}")
                # column-split across both HWDGE rings: halves drain in
                # parallel, and the low half (which the first products read)
                # lands first
                H = W // 2
                er, ei = (nc.sync, nc.scalar) if b == 0 else (nc.scalar, nc.sync)
                o_r = (b * 2) * DS_LEN + 384
                o_i = (b * 2 + 1) * DS_LEN + 384
                er.dma_start(wsr[:, 0:H], bass.AP(ds2, o_r, [[1, 128], [1, H]]))
                ei.dma_start(wsi[:, 0:H], bass.AP(ds2, o_i, [[1, 128], [1, H]]))
                er.dma_start(wsi[:, H:W], bass.AP(ds2, o_i + H, [[1, 128], [1, W - H]]))
                ei.dma_start(wsr[:, H:W], bass.AP(ds2, o_r + H, [[1, 128], [1, W - H]]))
                s["ws"] = (wsr, wsi)
                return s

            def emit_warm(s, n):
                # dummy matmuls to hold the PE HAM unthrottled before the real
                # matmul stream starts; reads the window tile, writes a scratch
                # PSUM bank.
                wsr, _ = s["ws"]
                ap = wsr[:]
                l = bass.AP(ap.tensor, ap.offset, [ap.ap[0], [1, 128]])
                r = bass.AP(ap.tensor, ap.offset, [ap.ap[0], [1, 512]])
                for _ in range(n):
                    nc.tensor.matmul(warm[:, 0:512], l, r, start=True, stop=True)

            def emit_keepalive(dep_tile, mov_tile=None):
                # dummy matmul whose stationary reads a freshly-written tile,
                # so it lands spaced through the producing stream and keeps
                # the PE HAM active
                ap = dep_tile[:]
                l = bass.AP(ap.tensor, ap.offset, [ap.ap[0], [1, 128]])
                mp = (mov_tile if mov_tile is not None else TTP[0])[:]
                r = bass.AP(mp.tensor, mp.offset, [mp.ap[0], [1, 512]])
                nc.tensor.matmul(warm[:, 0:512], l, r, start=True, stop=True)

            def emit_alpha_act(b, s):
                # alpha = 1/sum_m |s[m]|^2: every partition of the sliding
                # window holds a full period, so a free-axis square-accumulate
                # over any 1024 columns yields the global sum per partition.
                wsr, wsi = s["ws"]
                scr = tmpp.tile([128, 1024], f32, tag=f"asc{b % 2}")
                accR = smp.tile([128, 1], f32, tag=f"accR{b}")
                accI = smp.tile([128, 1], f32, tag=f"accI{b}")
                for wt, acc in ((wsr, accR), (wsi, accI)):
                    ap = wt[:]
                    src = bass.AP(ap.tensor, ap.offset + 512, [ap.ap[0], [1, 1024]])
                    nc.scalar.activation(scr[:], src, ACTF.Square, accum_out=acc[:])
                s["accs"] = (accR, accI)

            def emit_alpha_dve(b, s):
                accR, accI = s["accs"]
                ssum = smp.tile([128, 1], f32, tag=f"ssum{b}")
                nc.vector.tensor_add(ssum[:], accR[:], accI[:])
                alpha = smp.tile([128, 1], f32, tag=f"alpha{b}")
                nc.vector.reciprocal(alpha[:], ssum[:])
                s["alpha"] = alpha

            def emit_rbuild(s, lo, hi, keepalive=False):
                # R^T[m, kk] = s[m]*conj(s)[(m-kk)%N]; sum/diff of halves m and
                # m+512, written as fp8 into DoubleRow chunk-pair tiles.
                wsr, wsi = s["ws"]
                wapr, wapi = wsr[:], wsi[:]
                scol = s["scol"]
                b = s["b"]
                n = hi - lo
                for q in range(4):
                    us = []
                    for h in (0, 1):
                        c8 = q + 4 * h
                        sr_c = scol[:, c8:c8 + 1]
                        si_c = scol[:, 8 + c8:9 + c8]
                        nsr_c = scol[:, 16 + c8:17 + c8]
                        j0 = 640 + 128 * q + 512 * h - lo
                        w_sr = bass.AP(wapr.tensor, wapr.offset + j0, [wapr.ap[0], [-1, n]])
                        w_si = bass.AP(wapi.tensor, wapi.offset + j0, [wapi.ap[0], [-1, n]])
                        teng = nc.vector
                        a = tmpp.tile([128, 640], bf16, tag="ta")
                        ur = up.tile([128, 640], bf16, tag=f"ur{h}")
                        # Rr = sr_m*sr_win + si_m*si_win
                        teng.tensor_scalar_mul(a[:, 0:n], w_sr, sr_c)
                        nc.vector.scalar_tensor_tensor(
                            ur[:, 0:n], w_si, si_c, a[:, 0:n], op0=ALU.mult, op1=ALU.add)
                        b2 = tmpp.tile([128, 640], bf16, tag="tb")
                        ui = up.tile([128, 640], bf16, tag=f"ui{h}")
                        # Ri = si_m*sr_win - sr_m*si_win
                        teng.tensor_scalar_mul(b2[:, 0:n], w_si, nsr_c)
                        nc.vector.scalar_tensor_tensor(
                            ui[:, 0:n], w_sr, si_c, b2[:, 0:n], op0=ALU.mult, op1=ALU.add)
                        if keepalive:
                            emit_keepalive(ui, mov_tile=wsr)
                        us.append((ur, ui))
                    (u1r, u1i), (u2r, u2i) = us
                    qp, sub = q % 2, q // 2
                    if lo == 0:
                        for nm in ("rsr", "rsi", "rdr", "rdi"):
                            if (nm, qp) not in s["R"]:
                                s["R"][(nm, qp)] = rp.tile(
                                    [128, 2, 640], fp8,
                                    name=f"{nm}{qp}b{b}", tag=f"{nm}{qp}_{b % 2}",
                                )
                    def rsl(nm):
                        ap = s["R"][(nm, qp)][:]
                        return bass.AP(ap.tensor, ap.offset + sub * 640 + lo, [ap.ap[0], [1, n]])
                    nc.vector.tensor_add(rsl("rsr"), u1r[:, 0:n], u2r[:, 0:n])
                    nc.vector.tensor_sub(rsl("rdr"), u1r[:, 0:n], u2r[:, 0:n])
                    nc.vector.tensor_add(rsl("rsi"), u1i[:, 0:n], u2i[:, 0:n])
                    nc.vector.tensor_sub(rsl("rdi"), u1i[:, 0:n], u2i[:, 0:n])
                    if keepalive:
                        emit_keepalive(s["R"][("rdi", qp)])

            def rweights(s, nm, qp, c):
                ap = s["R"][(nm, qp)][:]
                return bass.AP(ap.tensor, ap.offset + c, [ap.ap[0], [640, 2], [1, 128]])

            def emit_kblock(b, s, kb):
                c = 128 * kb
                xe = psp.tile([128, 1024], f32, tag="xe")
                xo = psp.tile([128, 1024], f32, tag="xo")
                xre, xie = xe[:, 0:512], xe[:, 512:1024]
                xro, xio = xo[:, 0:512], xo[:, 512:1024]
                for qp in range(2):
                    first = qp == 0
                    last = qp == 1
                    psr = rweights(s, "rsr", qp, c)
                    psi = rweights(s, "rsi", qp, c)
                    pdr = rweights(s, "rdr", qp, c)
                    pdi = rweights(s, "rdi", qp, c)
                    mm = lambda o, l, r, st, sp: nc.tensor.matmul(
                        o, l, r, start=st, stop=sp, perf_mode=PM.DoubleRow)
                    # xe's groups close first so its square starts while xo's
                    # last matmuls still stream
                    mm(xre, psr, tab("tec", qp), first, False)
                    mm(xie, psi, tab("tec", qp), first, False)
                    mm(xre, psi, tab("tes", qp), False, last)
                    mm(xie, psr, tab("tesn", qp), False, last)
                    mm(xro, pdr, tab("toc", qp), first, False)
                    mm(xio, pdi, tab("toc", qp), first, False)
                    mm(xro, pdi, tab("tos", qp), False, last)
                    mm(xio, pdr, tab("tosn", qp), False, last)

                chi_t = chip.tile([128, N], f32r, tag=f"chi{(5 * b + kb) % 6}")
                alpha = s["alpha"]
                for parity, x2 in ((0, xe), (1, xo)):
                    sq = tmpp.tile([128, 1024], f32, tag=f"sq{parity}")
                    # chi = (alpha*xr)^2 + (alpha*xi)^2: normalization folded
                    # into the activation scale
                    nc.scalar.activation(sq[:], x2[:], ACTF.Square, scale=alpha[:])
                    cap = chi_t[:]
                    strided = bass.AP(cap.tensor, cap.offset + parity, [cap.ap[0], [2, 512]])
                    aeng = nc.gpsimd if b == 0 else nc.vector
                    aeng.tensor_add(strided, sq[:, 0:512], sq[:, 512:1024])
                s["chis"].append(chi_t)

            def emit_direct(b, s, kbs):
                for kb in kbs:
                    r0 = (128 * kb + 512) % N
                    ch = s["chis"][kb][:].bitcast(f32)
                    if b == 1 and kb == 4:
                        nc.sync.dma_start(out[b, r0:r0 + 64, :], s["chis"][kb][0:64, :].bitcast(f32))
                        nc.gpsimd.dma_start(out[b, r0 + 64:r0 + 128, :], s["chis"][kb][64:128, :].bitcast(f32))
                    else:
                        nc.sync.dma_start(out[b, r0:r0 + 128, :], ch)

            def emit_mirror_jcopy(b, s, kbs):
                # k-flip: J matmul on chi reverses partitions; the f-reversal
                # rides the PSUM->SBUF copies. ACT handles batch 0 (DVE busy
                # with rbuild(s1)); DVE handles batch 1 (idle by then).
                s.setdefault("mj", {})
                cp = nc.scalar.copy
                for kb in kbs:
                    chi_t = s["chis"][kb]
                    jy = psp.tile([128, 1024], f32, tag="xe")
                    nc.tensor.matmul(jy[:, 0:512], tJ[:],
                                     chi_t[:, 0:512], start=True, stop=True)
                    nc.tensor.matmul(jy[:, 512:1024], tJ[:],
                                     chi_t[:, 512:1024], start=True, stop=True)
                    mj = mjp.tile([128, N], f32, tag=f"mj{kb % 2}")
                    jap = jy[:]
                    rev_hi = bass.AP(jap.tensor, jap.offset + 1023, [jap.ap[0], [-1, 511]])
                    rev_lo = bass.AP(jap.tensor, jap.offset + 511, [jap.ap[0], [-1, 511]])
                    cp(mj[:, 0:1], jy[:, 0:1])
                    cp(mj[:, 1:512], rev_hi)
                    cp(mj[:, 512:513], jy[:, 512:513])
                    cp(mj[:, 513:1024], rev_lo)
                    s["mj"][kb] = mj

            def emit_mirror_store(b, s, kbs):
                # mj partition r holds k = c+127-r -> dest row 385-c+r
                for kb in kbs:
                    c = 128 * kb
                    mj = s["mj"][kb]
                    eng = nc.scalar
                    if kb == 0:
                        eng.dma_start(out[b, 385:512, :], mj[0:127, :])
                    elif kb == 3:
                        eng.dma_start(out[b, 128:129, :], mj[127:128, :])
                    else:
                        r0 = 385 - c
                        if b == 1 and kb == 2:
                            # final mirror: split across rings so the tail
                            # drains in parallel
                            nc.scalar.dma_start(out[b, r0:r0 + 64, :], mj[0:64, :])
                            nc.gpsimd.dma_start(out[b, r0 + 64:r0 + 128, :], mj[64:128, :])
                        else:
                            eng.dma_start(out[b, r0:r0 + 128, :], mj[:])

            # --- pipelined schedule: DVE runs the two rbuilds back-to-back;
            # the PE streams batch-0 kblocks against rbuild(s1); warm/keepalive
            # matmuls bridge the PE-idle stretches so the HAM stays at 8/8.
            s0 = emit_load(0)
            nc.scalar.dma_start(tJ[:], jmat[:])
            s1 = emit_load(1)
            emit_warm(s0, 14)
            emit_alpha_act(0, s0)
            emit_alpha_act(1, s1)
            emit_rbuild(s0, 0, 640, keepalive=True)
            emit_alpha_dve(0, s0)
            emit_kblock(0, s0, 0)
            emit_direct(0, s0, [0])
            emit_alpha_dve(1, s1)
            emit_rbuild(s1, 0, 640)
            emit_kblock(0, s0, 1)
            emit_direct(0, s0, [1])
            emit_mirror_jcopy(0, s0, [0])
            emit_mirror_store(0, s0, [0])
            emit_kblock(0, s0, 2)
            emit_direct(0, s0, [2])
            emit_mirror_jcopy(0, s0, [1])
            emit_mirror_store(0, s0, [1])
            emit_kblock(0, s0, 3)
            emit_direct(0, s0, [3])
            emit_mirror_jcopy(0, s0, [2])
            emit_mirror_store(0, s0, [2])
            emit_kblock(0, s0, 4)
            emit_direct(0, s0, [4])
            emit_mirror_jcopy(0, s0, [3])
            emit_mirror_store(0, s0, [3])
            # bridge the gap until rbuild(s1) completes, one keepalive per
            # R array so the deps land spread across its combine stream
            for nm in ("rsr", "rsi", "rdr", "rdi"):
                ap = s1["R"][(nm, 1)][:]
                l = bass.AP(ap.tensor, ap.offset, [ap.ap[0], [1, 128]])
                tp_ap = TTP[0][:]
                r = bass.AP(tp_ap.tensor, tp_ap.offset, [tp_ap.ap[0], [1, 512]])
                nc.tensor.matmul(warm[:, 0:512], l, r, start=True, stop=True)
            emit_kblock(1, s1, 0)
            emit_direct(1, s1, [0])
            for kb in range(1, 4):
                emit_kblock(1, s1, kb)
                emit_direct(1, s1, [kb])
                emit_mirror_jcopy(1, s1, [kb - 1])
                emit_mirror_store(1, s1, [kb - 1])
            emit_kblock(1, s1, 4)
            emit_direct(1, s1, [4])
            emit_mirror_jcopy(1, s1, [3])
            emit_mirror_store(1, s1, [3])

    _split_excess_waits(nc)
    return nc


_NC_CACHE = {}


def _get_nc():
    if "nc" not in _NC_CACHE:
        _NC_CACHE["nc"] = build_nc()
    return _NC_CACHE["nc"]


def _get_tables():
    if "tabs" not in _NC_CACHE:
        m = np.arange(512, dtype=np.float64)[:, None]
        tp_ = np.arange(512, dtype=np.float64)[None, :]
        t_of = (tp_ + 256) % 512
        ang_e = 2.0 * np.pi * ((m * t_of) % 512) / 512
        ang_o = ang_e + 2.0 * np.pi * m / 1024
        tabs = {
            "tec": np.cos(ang_e),
            "tes": np.sin(ang_e),
            "toc": np.cos(ang_o),
            "tos": np.sin(ang_o),
        }
        tabs["tesn"] = -tabs["tes"]
        tabs["tosn"] = -tabs["tos"]
        # paired fp8 layout: tabsp[qp, p, sub, 512*i+t] = tab_i[128*(qp+2*sub)+p, t]
        tabsp = np.zeros((2, 128, 2, 6 * 512), dtype=np.float64)
        for i, nm in enumerate(TABNAMES):
            tq = tabs[nm].reshape(4, 128, 512)  # [chunk, p, t]
            for qp in range(2):
                for sub in range(2):
                    tabsp[qp, :, sub, 512 * i:512 * (i + 1)] = tq[qp + 2 * sub]
        _NC_CACHE["tabs"] = (
            tabsp.astype(ml_dtypes.float8_e4m3),
            np.eye(128, dtype=np.float32)[::-1].copy(),
        )
    return _NC_CACHE["tabs"]


def make_in_maps(s_real: np.ndarray, s_imag: np.ndarray):
    s_real = np.asarray(s_real, dtype=np.float32)
    s_imag = np.asarray(s_imag, dtype=np.float32)
    tabsp, jnp_ = _get_tables()
    in_maps = []
    for core in range(NCORES):
        sl = slice(core * BPC, (core + 1) * BPC)
        sr = s_real[sl].astype(ml_dtypes.bfloat16)
        si = s_imag[sl].astype(ml_dtypes.bfloat16)
        ds2 = np.stack(
            [np.tile(sr, (1, 3))[:, :DS_LEN], np.tile(si, (1, 3))[:, :DS_LEN]],
            axis=1,
        ).copy()
        scols = np.concatenate(
            [
                sr.reshape(BPC, 8, 128).transpose(0, 2, 1),
                si.reshape(BPC, 8, 128).transpose(0, 2, 1),
                (-sr).reshape(BPC, 8, 128).transpose(0, 2, 1),
            ],
            axis=2,
        ).astype(np.float32).copy()
        im = {"ds2": ds2, "scols": scols, "tabsp": tabsp, "jmat": jnp_}
        in_maps.append(im)
    return in_maps


def kernel(s_real: np.ndarray, s_imag: np.ndarray) -> np.ndarray:
    nc = _get_nc()
    in_maps = make_in_maps(s_real, s_imag)
    res = bass_utils.run_bass_kernel_spmd(nc, in_maps, core_ids=list(range(NCORES)))
    return np.concatenate([r["out"] for r in res.results], axis=0)
